# revision 1
# baseline (speedup 1.0000x reference)
"""Couplformer attention kernel, data-parallel across 8 NeuronCores.

Shapes (hardcoded): x [16, 4096, 384], W_qkv [1152, 384], b_qkv [1152],
W_proj [384, 384], b_proj [384].  B=16 is sharded 2-per-core across the
8 cores; every op (qkv proj, height/width attention, out proj) is
independent per batch element, so no collectives are needed.
"""

import numpy as np

B, N, C = 16, 4096, 384
NH, HD = 12, 32
HT, WD = 64, 64
SCALE = HD ** (-0.25)
NCORES = 8
BL = B // NCORES  # batches per core


def _couplformer_local(x, W_qkv, b_qkv, W_proj, b_proj, jnp):
    """Per-shard computation: x is [BL, N, C]."""
    qkv = x @ W_qkv.T + b_qkv
    qkv = (
        qkv.reshape(BL, N, 3, NH, HD)
        .transpose(2, 0, 3, 1, 4)
        .reshape(3, BL, NH, HT, WD, HD)
    )
    q, k, v = qkv[0], qkv[1], qkv[2]

    a = jnp.einsum("bhywc,bhzwc->bhyz", q, k) * SCALE
    a = jax_softmax(a, jnp)

    b_attn = jnp.einsum("bhywc,bhyvc->bhwv", q, k) * SCALE
    b_attn = jax_softmax(b_attn, jnp)

    out1 = jnp.einsum("bhywc,bhvw->bhcyv", v, b_attn)
    out = jnp.einsum("bhuy,bhcyv->bhcuv", a, out1)

    out = out.reshape(BL, C, N).transpose(0, 2, 1)
    out = out @ W_proj.T + b_proj
    return out


def jax_softmax(logits, jnp):
    m = jnp.max(logits, axis=-1, keepdims=True)
    e = jnp.exp(logits - m)
    return e / jnp.sum(e, axis=-1, keepdims=True)


_PMAP_FN = None


def _get_pmap_fn():
    global _PMAP_FN
    if _PMAP_FN is None:
        import jax
        import jax.numpy as jnp

        devs = jax.devices()[:NCORES]

        def shard_fn(x, W_qkv, b_qkv, W_proj, b_proj):
            return _couplformer_local(x, W_qkv, b_qkv, W_proj, b_proj, jnp)

        _PMAP_FN = jax.pmap(
            shard_fn,
            in_axes=(0, None, None, None, None),
            devices=devs,
        )
    return _PMAP_FN


def kernel(x, W_qkv, b_qkv, W_proj, b_proj):
    x = np.asarray(x, dtype=np.float32)
    W_qkv = np.asarray(W_qkv, dtype=np.float32)
    b_qkv = np.asarray(b_qkv, dtype=np.float32)
    W_proj = np.asarray(W_proj, dtype=np.float32)
    b_proj = np.asarray(b_proj, dtype=np.float32)

    try:
        fn = _get_pmap_fn()
        xs = x.reshape(NCORES, BL, N, C)
        out = fn(xs, W_qkv, b_qkv, W_proj, b_proj)
        out = np.asarray(out).reshape(B, N, C)
        return out.astype(np.float32)
    except Exception:
        # CPU fallback: same math in numpy (correct, not accelerated).
        out = np.empty((B, N, C), dtype=np.float32)
        for b0 in range(B):
            out[b0] = _couplformer_local(
                x[b0 : b0 + 1], W_qkv, b_qkv, W_proj, b_proj, np
            )[0]
        return out



# revision 7
# speedup vs baseline: 5094.2929x; 5094.2929x over previous
"""Couplformer attention kernel for 8 Trainium2 NeuronCores (Bass/Tile).

Shapes: x [16, 4096, 384], W_qkv [1152, 384], b_qkv [1152],
W_proj [384, 384], b_proj [384].  Data-parallel: B=16 -> 2 batches/core.

Per-core pipeline (bf16 compute, fp32 PSUM):
  1. xT via DRAM->SBUF DMA-transpose.
  2. token-major qkv projection (lhsT = xT chunks, rhs = W_qkv^T).
  3. scatter-write qkv to DRAM in two fold layouts:
       l1d [y, w, (h,c)] (2.3KB runs), l4d [w, y, (h,c)_qk] (1.5KB runs).
  4. per 4-head group: staged reads of L1q/L1k/L1v/L4q/L4k slices.
  5. per head: DVE/ACT slice-copies -> packed [128, 4096] tile -> ONE
     DMA-transpose -> 32 contraction chunks [128, 64|64] for both
     height-logits (L2 = [(w,c), y]) and width-logits (L3 = [(y,c), w]).
  6. chunked logits matmuls (16 x K=128 N=64, PSUM-accumulated), softmax
     via ACT exp (scale=HD^-0.25) with fused row-sum, PE-transpose of A/B.
  7. MM#1: T2[u,(j,c)] = A^T-weighted sum of V (strided rhs, N=512).
  8. group fold W1 -> DRAM -> W1r [j, (u,h,c)]; MM#2 with B^T.
  9. fold attention output -> channel-major X_cm [(h,c), (i,u)].
 10. token-major output projection; scatter-write final tokens (768B runs).

Biases are applied exactly via K=1 ones-row matmuls.
"""

import numpy as np

B, N, C = 16, 4096, 384
NH, HD = 12, 32
HT, WD = 64, 64
SCALE = float(HD ** (-0.25))
NCORES = 8
BL = B // NCORES  # 2 batches per core
HG = 4            # heads per group
NG = NH // HG     # 3 groups

_CACHE = {}


def _build_nc():
    import concourse.bacc as bacc
    import concourse.mybir as mybir
    from concourse import tile
    from concourse.masks import make_identity

    BF16 = mybir.dt.bfloat16
    F32 = mybir.dt.float32
    AF = mybir.ActivationFunctionType

    nc = bacc.Bacc("TRN2", target_bir_lowering=False, debug=False,
                   enable_asserts=False, num_devices=NCORES)

    x_d = nc.dram_tensor("x_d", [BL * N, C], BF16, kind="ExternalInput").ap()
    wq_d = nc.dram_tensor("wq_d", [C, 3 * C], BF16, kind="ExternalInput").ap()
    wp_d = nc.dram_tensor("wp_d", [C, C], BF16, kind="ExternalInput").ap()
    bq_d = nc.dram_tensor("bq_d", [1, 3 * C], BF16, kind="ExternalInput").ap()
    bp_d = nc.dram_tensor("bp_d", [1, C], BF16, kind="ExternalInput").ap()
    out_d = nc.dram_tensor("out_d", [BL * N, C], BF16, kind="ExternalOutput").ap()

    with tile.TileContext(nc) as tc:
        with (
            tc.tile_pool(name="const", bufs=1) as constp,
            tc.tile_pool(name="xtp", bufs=1) as xtp,
            tc.tile_pool(name="tmp", bufs=4) as tmp,
            tc.tile_pool(name="grp", bufs=1) as grp,
            tc.tile_pool(name="headp", bufs=2) as headp,
            tc.tile_pool(name="abp", bufs=2) as abp,
            tc.tile_pool(name="smallp", bufs=4) as smallp,
            tc.tile_pool(name="btp", bufs=8) as btp,
            tc.tile_pool(name="xcmp", bufs=1) as xcmp,
            tc.tile_pool(name="outp", bufs=4) as outp,
            tc.tile_pool(name="ps", bufs=8, space="PSUM") as ps,
            tc.tile_pool(name="dram", bufs=1, space="DRAM") as dram,
        ):
            # ---- constants
            wq = constp.tile([128, 3, 3 * C], BF16)   # [cin%128, cc, cout]
            nc.sync.dma_start(wq[:], wq_d.rearrange("(cc p) co -> p cc co", p=128))
            wp = constp.tile([128, 3, C], BF16)
            nc.sync.dma_start(wp[:], wp_d.rearrange("(cc p) co -> p cc co", p=128))
            bq = constp.tile([1, 3 * C], BF16)
            nc.sync.dma_start(bq[:], bq_d)
            bp = constp.tile([1, C], BF16)
            nc.sync.dma_start(bp[:], bp_d)
            ident = constp.tile([128, 128], BF16)
            make_identity(nc, ident[:])
            ones = constp.tile([1, 128], BF16)
            nc.gpsimd.memset(ones[:], 1.0)

            for b in range(BL):
                # ---- DRAM staging for this batch
                l1d = dram.tile([HT, WD, 3 * C], BF16, tag="l1d")
                l4d = dram.tile([WD, HT, 2 * C], BF16, tag="l4d")

                # ---- 1. xT [cin, tok] via DMA-transpose
                xt = xtp.tile([128, 3, N], BF16, tag="xt")
                for cc in range(3):
                    nc.sync.dma_start(
                        xt[:, cc, :],
                        x_d[b * N:(b + 1) * N, cc * 128:(cc + 1) * 128],
                        transpose=True,
                    )

                # ---- 2+3. qkv projection (token-major) + scatter-writes
                for t in range(32):
                    tmt = tmp.tile([128, 3 * C], BF16, tag="tmt")
                    for co in range(3):
                        acc = ps.tile([128, 384], F32, tag="pp")
                        for cc in range(3):
                            nc.tensor.matmul(
                                acc[:],
                                xt[:, cc, t * 128:(t + 1) * 128],
                                wq[:, cc, co * 384:(co + 1) * 384],
                                start=(cc == 0), stop=False,
                            )
                        nc.tensor.matmul(
                            acc[:], ones[:, 0:128],
                            bq[:, co * 384:(co + 1) * 384],
                            start=False, stop=True,
                        )
                        eng = nc.vector if (t + co) % 2 == 0 else nc.scalar
                        if eng is nc.vector:
                            eng.tensor_copy(tmt[:, co * 384:(co + 1) * 384], acc[:])
                        else:
                            eng.activation(tmt[:, co * 384:(co + 1) * 384], acc[:], AF.Copy)
                    # scatter-writes: rows of tile t are tok = t*128 + p,
                    # y = 2t + p//64, w = p%64
                    nc.sync.dma_start(
                        l1d.rearrange("y w hc -> (y w) hc")[t * 128:(t + 1) * 128, :],
                        tmt[:],
                    )
                    nc.scalar.dma_start(l4d[:, 2 * t, :], tmt[0:64, 0:768])
                    nc.scalar.dma_start(l4d[:, 2 * t + 1, :], tmt[64:128, 0:768])

                xcm = [
                    xcmp.tile([128, N], BF16, tag=f"xcm{cc}", name=f"xcm{cc}")
                    for cc in range(3)
                ]

                for g in range(NG):
                    # ---- 4. group reads (q -> parts 0:64, k -> parts 64:128)
                    t1 = grp.tile([128, WD, HG * HD], BF16, tag="t1")  # L1q|L1k
                    nc.sync.dma_start(t1[0:64], l1d[:, :, g * 128:(g + 1) * 128])
                    nc.sync.dma_start(t1[64:128], l1d[:, :, 384 + g * 128:384 + (g + 1) * 128])
                    t2 = grp.tile([128, HT, HG * HD], BF16, tag="t2")  # L4q|L4k
                    nc.scalar.dma_start(t2[0:64], l4d[:, :, g * 128:(g + 1) * 128])
                    nc.scalar.dma_start(t2[64:128], l4d[:, :, 384 + g * 128:384 + (g + 1) * 128])
                    tv = grp.tile([64, WD, HG * HD], BF16, tag="tv")   # L1v
                    nc.sync.dma_start(tv[:], l1d[:, :, 768 + g * 128:768 + (g + 1) * 128])

                    w1all = grp.tile([64, WD, HG, HD], BF16, tag="w1all")
                    bts = []

                    for hh in range(HG):
                        # ---- 5. slice-copies into packed tile + DMA-T
                        qk = headp.tile([128, 2, WD * HD], BF16, tag="qk")
                        nc.vector.tensor_copy(
                            qk[0:64, 0, :].rearrange("p (w c) -> p w c", c=HD),
                            t1.rearrange("p w (hh c) -> p w hh c", hh=HG)[0:64, :, hh, :],
                        )
                        nc.vector.tensor_copy(
                            qk[64:128, 0, :].rearrange("p (w c) -> p w c", c=HD),
                            t1.rearrange("p w (hh c) -> p w hh c", hh=HG)[64:128, :, hh, :],
                        )
                        nc.vector.tensor_copy(
                            qk[0:64, 1, :].rearrange("p (w c) -> p w c", c=HD),
                            t2.rearrange("p w (hh c) -> p w hh c", hh=HG)[0:64, :, hh, :],
                        )
                        nc.scalar.copy(
                            qk[64:128, 1, :].rearrange("p (w c) -> p w c", c=HD),
                            t2.rearrange("p w (hh c) -> p w hh c", hh=HG)[64:128, :, hh, :],
                        )
                        ab = abp.tile([128, 32, 128], BF16, tag="ab")
                        nc.sync.dma_start(ab[:], qk.rearrange("p a f -> p (a f)"), transpose=True)

                        # ---- 6. logits + softmax + PE-T  (a: t 0:16, b: t 16:32)
                        mats = []
                        for s in range(2):
                            lg = ps.tile([64, 64], F32, tag="pp")
                            for t in range(16):
                                nc.tensor.matmul(
                                    lg[:],
                                    ab[:, s * 16 + t, 0:64],
                                    ab[:, s * 16 + t, 64:128],
                                    start=(t == 0), stop=(t == 15),
                                )
                            exps = smallp.tile([64, 64], F32, tag="exps")
                            ssum = smallp.tile([64, 1], F32, tag="ssum")
                            nc.scalar.activation(exps[:], lg[:], AF.Exp,
                                                 scale=SCALE, accum_out=ssum[:])
                            rsum = smallp.tile([64, 1], F32, tag="rsum")
                            nc.vector.reciprocal(rsum[:], ssum[:])
                            amat = smallp.tile([64, 64], BF16, tag="amat")
                            nc.vector.tensor_scalar_mul(amat[:], exps[:], rsum[:])
                            # PE transpose -> [key, query] layout for lhsT use
                            tps = ps.tile([64, 64], BF16, tag="pp")
                            nc.tensor.transpose(tps[:], amat[:], ident[0:64, 0:64])
                            if s == 0:
                                tmat = smallp.tile([64, 64], BF16, tag="atb", name="atb")
                            else:
                                tmat = btp.tile([64, 64], BF16, tag="btb", name="btb")
                            nc.vector.tensor_copy(tmat[:], tps[:])
                            mats.append(tmat)
                        atb, btb = mats
                        bts.append(btb)

                        # ---- 7. MM#1: W1[u, (j,c)] chunks of N=512
                        for q4 in range(4):
                            w1p = ps.tile([64, 512], F32, tag="pp")
                            nc.tensor.matmul(
                                w1p[:], atb[:],
                                tv.rearrange("p w (hh c) -> p w hh c", hh=HG)[
                                    :, q4 * 16:(q4 + 1) * 16, hh, :],
                                start=True, stop=True,
                            )
                            eng = nc.vector if q4 % 2 == 0 else nc.scalar
                            dst = w1all[:, q4 * 16:(q4 + 1) * 16, hh, :]
                            src = w1p.rearrange("p (j c) -> p j c", c=HD)
                            if eng is nc.vector:
                                eng.tensor_copy(dst, src)
                            else:
                                eng.activation(dst, src, AF.Copy)

                    # ---- 8. group fold W1 -> DRAM -> W1r, then MM#2
                    w1d = dram.tile([64, WD * HG * HD], BF16, tag="w1d")
                    nc.sync.dma_start(w1d[:], w1all.rearrange("p j hh c -> p (j hh c)"))
                    w1r = grp.tile([64, 64, HG, HD], BF16, tag="w1r")  # [j, u, h, c]
                    nc.sync.dma_start(
                        w1r.rearrange("j u hh c -> j u (hh c)"),
                        w1d.rearrange("u (j hc) -> j u hc", j=WD),
                    )
                    o2all = grp.tile([64, HG, HD, 64], BF16, tag="o2all")  # [i,h,c,u]
                    for hh in range(HG):
                        for q4 in range(4):
                            op = ps.tile([64, 512], F32, tag="pp")
                            nc.tensor.matmul(
                                op[:], bts[hh][:],
                                w1r[:, q4 * 16:(q4 + 1) * 16, hh, :],
                                start=True, stop=True,
                            )
                            eng = nc.vector if (hh + q4) % 2 == 0 else nc.scalar
                            dst = o2all[:, hh, :, q4 * 16:(q4 + 1) * 16]
                            src = op.rearrange("p (u c) -> p c u", c=HD)
                            if eng is nc.vector:
                                eng.tensor_copy(dst, src)
                            else:
                                eng.activation(dst, src, AF.Copy)

                    # ---- 9. fold O2 -> DRAM -> X_cm rows
                    o2d = dram.tile([64, HG * HD * 64], BF16, tag="o2d")
                    nc.scalar.dma_start(o2d[:], o2all.rearrange("p hh c u -> p (hh c u)"))
                    for hh in range(HG):
                        h = g * HG + hh
                        cc, r0 = h // 4, (h % 4) * 32
                        nc.sync.dma_start(
                            xcm[cc][r0:r0 + 32].rearrange("p (i u) -> p i u", i=64),
                            o2d.rearrange("i (hh c u) -> hh c i u", hh=HG, c=HD)[hh],
                        )

                # ---- 10. output projection (token-major) + scattered write
                for t in range(32):
                    acc = ps.tile([128, C], F32, tag="pp")
                    for cc in range(3):
                        nc.tensor.matmul(
                            acc[:],
                            xcm[cc][:, t * 128:(t + 1) * 128],
                            wp[:, cc, :],
                            start=(cc == 0), stop=False,
                        )
                    nc.tensor.matmul(acc[:], ones[:, 0:128], bp[:],
                                     start=False, stop=True)
                    ot = outp.tile([128, C], BF16, tag="ot")
                    if t % 2 == 0:
                        nc.vector.tensor_copy(ot[:], acc[:])
                    else:
                        nc.scalar.activation(ot[:], acc[:], AF.Copy)
                    # partitions = (i', u); token = u*64 + 2t + i'
                    for ii in range(2):
                        nc.sync.dma_start(
                            out_d.rearrange("(bb u w) c -> bb w u c", u=64, w=64)[
                                b, 2 * t + ii, :, :],
                            ot[ii * 64:(ii + 1) * 64, :],
                        )

    nc.compile()
    return nc


def _get_nc():
    if "nc" not in _CACHE:
        _CACHE["nc"] = _build_nc()
    return _CACHE["nc"]


def _run_on_hw(x, W_qkv, b_qkv, W_proj, b_proj, trace=False):
    import ml_dtypes
    from concourse.bass_utils import run_bass_kernel_spmd

    bf16 = ml_dtypes.bfloat16
    nc = _get_nc()
    wq = np.ascontiguousarray(W_qkv.T).astype(bf16)      # [384, 1152]
    wp = np.ascontiguousarray(W_proj.T).astype(bf16)     # [384, 384]
    bqv = b_qkv.reshape(1, -1).astype(bf16)
    bpv = b_proj.reshape(1, -1).astype(bf16)
    xs = x.reshape(NCORES, BL * N, C).astype(bf16)
    in_maps = [
        {"x_d": xs[c], "wq_d": wq, "bq_d": bqv, "wp_d": wp, "bp_d": bpv}
        for c in range(NCORES)
    ]
    res = run_bass_kernel_spmd(nc, in_maps, core_ids=list(range(NCORES)),
                               trace=trace)
    out = np.stack([res.results[c]["out_d"] for c in range(NCORES)])
    out = out.astype(np.float32).reshape(B, N, C)
    return out, res


def kernel(x, W_qkv, b_qkv, W_proj, b_proj):
    x = np.asarray(x, dtype=np.float32)
    W_qkv = np.asarray(W_qkv, dtype=np.float32)
    b_qkv = np.asarray(b_qkv, dtype=np.float32)
    W_proj = np.asarray(W_proj, dtype=np.float32)
    b_proj = np.asarray(b_proj, dtype=np.float32)
    out, _ = _run_on_hw(x, W_qkv, b_qkv, W_proj, b_proj, trace=False)
    return out
